# revision 33
# baseline (speedup 1.0000x reference)
"""Adaptive Computation Time step — Trainium2 Bass kernel.

Pure data parallel over batch B=8 on 8 NeuronCores (one batch row per core).

Per-core layouts (M=2048 tokens, H=1024, P=128, C=16; M == P*C):
  - "AT" layout: SBUF [16, 128], token m at [m // 128, m % 128].
    DRAM [2048] <-> AT is a contiguous 512B-per-partition DMA.
  - "A" layout: token m at [m % 128, m // 128] — matches the big H tile
    [128, 16, 1024] (token = partition, chunk = m // 128) used by the
    per-chunk indirect gathers/scatters, whose index APs are [128,1] columns.
  - AT <-> A is a single PE transpose (via identity matmul).

Algorithm per core:
  rank   = cumsum(run) - 1                  (DVE scan + 16x16 tri matmul)
  H[t]   = x[rank] for running tokens       (indirect row gather; non-running
           rows are OOB-skipped and the chunk is pre-zeroed: h = 0 there,
           exactly like the reference)
  p      = sigmoid(h @ p_w + p_b) * run     (fused mul+accum per chunk + ACT)
  mc     = (acc_p + p < 0.99) & run ; me = run & ~mc       (per half, A layout,
  update = p*mc + (1-acc_p)*me               so chunk FMAs start early)
  weighted_h_new = h*update + weighted_h    (one fused DVE op per chunk)
  c2     = cumsum(mc); dest = mc ? c2-1 : count_new + (m - c2)  (bijection)
  H[~mc rows] = pad_h (copy_predicated), then per-chunk indirect scatter of
  H rows to dest — every h_packed row is written exactly once.
"""

import functools
import sys

sys.path.insert(0, "/opt/trn_rl_repo")

import numpy as np

B, M, H = 8, 2048, 1024
P, C = 128, 16
TH = 0.99
NCORES = 8
OOB = 4096.0  # > M-1 -> skipped by bounds_check


@functools.lru_cache(maxsize=4)
def _build(updates_plus1: float):
    import concourse.bass as bass
    import concourse.tile as tile
    from concourse import bacc, mybir

    f32 = mybir.dt.float32
    i32 = mybir.dt.int32
    u8 = mybir.dt.uint8
    Alu = mybir.AluOpType
    Act = mybir.ActivationFunctionType

    nc = bacc.Bacc("TRN2", target_bir_lowering=False, debug=False)

    x = nc.dram_tensor("x", [M, H], f32, kind="ExternalInput")
    run8 = nc.dram_tensor("run8", [M], u8, kind="ExternalInput")
    accp = nc.dram_tensor("accp", [M], f32, kind="ExternalInput")
    wh = nc.dram_tensor("wh", [M, H], f32, kind="ExternalInput")
    rem = nc.dram_tensor("rem", [M], f32, kind="ExternalInput")
    exi = nc.dram_tensor("exi", [M], i32, kind="ExternalInput")
    pad128_d = nc.dram_tensor("pad128", [P, H], f32, kind="ExternalInput")
    pw128_d = nc.dram_tensor("pw128", [P, H], f32, kind="ExternalInput")
    pb128_d = nc.dram_tensor("pb128", [P, 1], f32, kind="ExternalInput")
    us16_d = nc.dram_tensor("us16", [C, C], f32, kind="ExternalInput")
    us128_d = nc.dram_tensor("us128", [P, P], f32, kind="ExternalInput")
    id16_d = nc.dram_tensor("id16", [C, C], f32, kind="ExternalInput")
    id128_d = nc.dram_tensor("id128", [P, P], f32, kind="ExternalInput")

    hp = nc.dram_tensor("hp", [M, H], f32, kind="ExternalOutput")
    whn = nc.dram_tensor("whn", [M, H], f32, kind="ExternalOutput")
    accn = nc.dram_tensor("accn", [M], f32, kind="ExternalOutput")
    remn = nc.dram_tensor("remn", [M], f32, kind="ExternalOutput")
    runn = nc.dram_tensor("runn", [M], u8, kind="ExternalOutput")
    exn = nc.dram_tensor("exn", [M], i32, kind="ExternalOutput")

    def at_view(dram):  # DRAM [2048] -> [16,128] AT view
        return dram.ap().rearrange("(t p) -> t p", p=P)

    with tile.TileContext(nc) as tc:
        with (
            tc.tile_pool(name="const", bufs=1) as cst,
            tc.tile_pool(name="small", bufs=1) as sm,
            tc.tile_pool(name="big", bufs=1) as big,
            tc.tile_pool(name="whs", bufs=16) as whs,
            tc.tile_pool(name="outs", bufs=4) as outs,
            tc.tile_pool(name="scr", bufs=2) as scr,
            tc.tile_pool(name="ps", bufs=2, space="PSUM") as ps,
            tc.tile_pool(name="ps2", bufs=2, space="PSUM") as ps2,
        ):
            # ---- critical-path loads on sync; the rest on scalar ----
            run8_t = sm.tile([C, P], u8)
            nc.sync.dma_start(out=run8_t[:], in_=at_view(run8))
            id16 = cst.tile([C, C], f32)
            nc.sync.dma_start(out=id16[:], in_=id16_d.ap())
            us16 = cst.tile([C, C], f32)
            nc.sync.dma_start(out=us16[:], in_=us16_d.ap())
            Ht = big.tile([P, C, H], f32)
            nc.scalar.memzero(Ht[:, 0:4, :])
            pad128 = cst.tile([P, H], f32)
            nc.scalar.dma_start(out=pad128[:], in_=pad128_d.ap())
            pw128 = cst.tile([P, H], f32)
            nc.scalar.dma_start(out=pw128[:], in_=pw128_d.ap())
            pb128 = cst.tile([P, 1], f32)
            nc.scalar.dma_start(out=pb128[:], in_=pb128_d.ap())
            accp_t = sm.tile([C, P], f32)
            nc.sync.dma_start(out=accp_t[:], in_=at_view(accp))
            id128 = cst.tile([P, P], f32)
            nc.scalar.dma_start(out=id128[:], in_=id128_d.ap())
            us128 = cst.tile([P, P], f32)
            nc.scalar.dma_start(out=us128[:], in_=us128_d.ap())
            zrow = cst.tile([P, C], f32)
            nc.vector.memset(zrow[:], 0.0)
            rem_t = sm.tile([C, P], f32)
            nc.sync.dma_start(out=rem_t[:], in_=at_view(rem))
            exi_t = sm.tile([C, P], i32)
            nc.sync.dma_start(out=exi_t[:], in_=at_view(exi))
            # remaining big-tile memzero quarters (quarter 0 was issued at
            # the top of the ACT stream; OOB-skipped rows must read 0.0)
            for q in range(1, 4):
                nc.scalar.memzero(Ht[:, q * 4 : (q + 1) * 4, :])

            runf = sm.tile([C, P], f32)
            nc.vector.tensor_copy(out=runf[:], in_=run8_t[:])
            z16 = sm.tile([C, P], f32)
            nc.vector.memset(z16[:], 0.0)

            # ---- cumsum #1: rank, then masked gather index (AT layout) ----
            s1 = sm.tile([C, P], f32)
            nc.vector.tensor_tensor_scan(
                out=s1[:], data0=runf[:], data1=z16[:], initial=0.0,
                op0=Alu.add, op1=Alu.add,
            )
            ex1 = ps2.tile([C, 1], f32, space="PSUM", tag="tmisc")
            nc.tensor.matmul(out=ex1[:], lhsT=us16[:], rhs=s1[:, P - 1 : P],
                             start=True, stop=True)
            c1 = sm.tile([C, P], f32)
            nc.vector.tensor_scalar(
                out=c1[:], in0=s1[:], scalar1=ex1[:], scalar2=1.0,
                op0=Alu.add, op1=Alu.subtract,
            )
            # gidx = run ? max(rank,0) : OOB  ==  (max(rank,0) - OOB)*run + OOB
            g0 = sm.tile([C, P], f32)
            nc.vector.tensor_single_scalar(out=g0[:], in_=c1[:], scalar=0.0,
                                           op=Alu.max)
            g1 = sm.tile([C, P], f32)
            nc.vector.scalar_tensor_tensor(
                out=g1[:], in0=g0[:], scalar=-OOB, in1=runf[:],
                op0=Alu.add, op1=Alu.mult,
            )
            gidx_at = sm.tile([C, P], f32)
            nc.vector.tensor_single_scalar(out=gidx_at[:], in_=g1[:],
                                           scalar=OOB, op=Alu.add)
            ptg = ps.tile([P, C], f32, space="PSUM", tag="t128")
            nc.tensor.transpose(ptg[:], gidx_at[:], id16[:])
            gidx_a = sm.tile([P, C], i32)
            nc.vector.tensor_copy(out=gidx_a[:], in_=ptg[:])

            # A-layout run/accp for the per-half mask math
            ptr_ = ps.tile([P, C], f32, space="PSUM", tag="t128")
            nc.tensor.transpose(ptr_[:], runf[:], id16[:])
            runf_a = sm.tile([P, C], f32)
            nc.scalar.copy(out=runf_a[:], in_=ptr_[:])
            pta = ps.tile([P, C], f32, space="PSUM", tag="t128")
            nc.tensor.transpose(pta[:], accp_t[:], id16[:])
            accp_a = sm.tile([P, C], f32)
            nc.scalar.copy(out=accp_a[:], in_=pta[:])

            # ---- prefill h_packed with pad rows (sequential writes that
            # overlap the read-heavy gather phase); the valid scatter later
            # overwrites the first count_new rows ----
            for t in range(C):
                nc.sync.dma_start(out=hp.ap()[t * P : (t + 1) * P, :],
                                  in_=pad128[:])

            # ---- weighted_h loads (early, deep pipeline) ----
            wh_tiles = []
            for t in range(C):
                wt = whs.tile([P, H], f32, tag="wht")
                nc.sync.dma_start(out=wt[:], in_=wh.ap()[t * P : (t + 1) * P, :])
                wh_tiles.append(wt)

            HALF = C // 2
            pdot = sm.tile([P, C], f32)
            sig_a = sm.tile([P, C], f32)
            p_a = sm.tile([P, C], f32)
            mc_a = sm.tile([P, C], f32)
            me_a = sm.tile([P, C], f32)
            u1_a = sm.tile([P, C], f32)
            u2_a = sm.tile([P, C], f32)
            upd_a = sm.tile([P, C], f32)
            accn_a = sm.tile([P, C], f32)

            def half_math(hf):
                hs = slice(hf * HALF, (hf + 1) * HALF)
                nc.scalar.activation(sig_a[:, hs], pdot[:, hs], Act.Sigmoid,
                                     bias=pb128[:], scale=1.0)
                nc.vector.tensor_mul(out=p_a[:, hs], in0=sig_a[:, hs],
                                     in1=runf_a[:, hs])
                tmp = sm.tile([P, HALF], f32, tag="tmp_a")
                nc.vector.tensor_add(out=tmp[:], in0=accp_a[:, hs],
                                     in1=p_a[:, hs])
                mcf = sm.tile([P, HALF], f32, tag="mcf_a")
                nc.vector.tensor_single_scalar(out=mcf[:], in_=tmp[:],
                                               scalar=TH, op=Alu.is_lt)
                nc.vector.tensor_mul(out=mc_a[:, hs], in0=mcf[:],
                                     in1=runf_a[:, hs])
                nc.vector.tensor_sub(out=me_a[:, hs], in0=runf_a[:, hs],
                                     in1=mc_a[:, hs])
                nc.vector.tensor_mul(out=u1_a[:, hs], in0=p_a[:, hs],
                                     in1=mc_a[:, hs])
                t2 = sm.tile([P, HALF], f32, tag="t2_a")
                nc.vector.tensor_mul(out=t2[:], in0=accp_a[:, hs],
                                     in1=me_a[:, hs])
                nc.vector.tensor_sub(out=u2_a[:, hs], in0=me_a[:, hs],
                                     in1=t2[:])
                nc.vector.tensor_add(out=upd_a[:, hs], in0=u1_a[:, hs],
                                     in1=u2_a[:, hs])
                nc.vector.tensor_add(out=accn_a[:, hs], in0=accp_a[:, hs],
                                     in1=u1_a[:, hs])

            # gathers + pdots chase each other; mask math per half
            for t in range(C):
                nc.gpsimd.indirect_dma_start(
                    out=Ht[:, t, :],
                    out_offset=None,
                    in_=x.ap(),
                    in_offset=bass.IndirectOffsetOnAxis(
                        ap=gidx_a[:, t : t + 1], axis=0),
                    bounds_check=M - 1,
                    oob_is_err=False,
                )
                sc = scr.tile([P, H], f32, tag="scr")
                nc.vector.scalar_tensor_tensor(
                    out=sc[:], in0=Ht[:, t, :], scalar=1.0, in1=pw128[:],
                    op0=Alu.mult, op1=Alu.mult,
                    accum_out=pdot[:, t : t + 1],
                )
                if t == HALF - 1:
                    half_math(0)
            half_math(1)


            # ---- cumsum #2 over mc, entirely in A layout (no transposes
            # on the critical path): partition-prefix via one 128x128
            # triangular matmul, column-prefix via a 16-wide scan on
            # partition 127 + a K=1 broadcast matmul ----
            colpref = ps.tile([P, C], f32, space="PSUM", tag="t128")
            nc.tensor.matmul(out=colpref[:], lhsT=us128[:], rhs=mc_a[:],
                             start=True, stop=True)
            colp_sb = sm.tile([P, C], f32)
            nc.scalar.copy(out=colp_sb[:], in_=colpref[:])
            # column totals to partition 0 (us128 col 127 is all-ones)
            ctot = ps2.tile([1, C], f32, space="PSUM", tag="tb")
            nc.tensor.matmul(out=ctot[:], lhsT=us128[:, P - 1 : P],
                             rhs=mc_a[:], start=True, stop=True)
            srow = sm.tile([1, C], f32)
            nc.vector.tensor_tensor_scan(
                out=srow[:], data0=ctot[:], data1=zrow[0:1, 0:C], initial=0.0,
                op0=Alu.add, op1=Alu.add,
            )
            tb = ps2.tile([P, C], f32, space="PSUM", tag="tb")
            nc.tensor.matmul(out=tb[:], lhsT=us128[0:1, :],
                             rhs=srow[:], start=True, stop=True)
            c2a = sm.tile([P, C], f32)
            nc.vector.tensor_copy(out=c2a[:, 0:1], in_=colp_sb[:, 0:1])
            nc.vector.tensor_add(out=c2a[:, 1:C], in0=colp_sb[:, 1:C],
                                 in1=tb[:, 0 : C - 1])
            # vidx = mc ? c2 - 1 : OOB  ==  (c2 - 1 - OOB)*mc + OOB
            v1 = sm.tile([P, C], f32)
            nc.vector.tensor_single_scalar(out=v1[:], in_=c2a[:],
                                           scalar=-(1.0 + OOB), op=Alu.add)
            v2 = sm.tile([P, C], f32)
            nc.vector.tensor_mul(out=v2[:], in0=v1[:], in1=mc_a[:])
            v3 = sm.tile([P, C], f32)
            nc.vector.tensor_single_scalar(out=v3[:], in_=v2[:], scalar=OOB,
                                           op=Alu.add)
            vidx_a = sm.tile([P, C], i32)
            vidx_inst = nc.vector.tensor_copy(out=vidx_a[:], in_=v3[:])

            # ---- pack scatters: valid rows only (non-running rows OOB-skip,
            # the pad prefill remains in the tail slots). Call 0 keeps the
            # Tile dep on the prefill writes; once that completed, later
            # calls are ordered behind it on the same engine, so the WAW
            # chain between scatter calls can be dropped (destinations are
            # distinct packed slots).
            for t in range(C):
                nc.gpsimd.indirect_dma_start(
                    out=hp.ap(),
                    out_offset=bass.IndirectOffsetOnAxis(
                        ap=vidx_a[:, t : t + 1], axis=0),
                    in_=Ht[:, t, :],
                    in_offset=None,
                    bounds_check=M - 1,
                    oob_is_err=False,
                )
                tc.dep_state.clear_tensor_accesses(hp.name)


            # ---- FMAs (weighted_h_new) on DVE. The no-sync edges keep the
            # greedy scheduler from interleaving them into the pdot stream /
            # mask math / pack-index chain, which gate the scatter stream.
            from concourse.tile import add_dep_helper as _adh
            for t in range(C):
                ot = outs.tile([P, H], f32, tag="outt")
                fi = nc.vector.scalar_tensor_tensor(
                    out=ot[:], in0=Ht[:, t, :],
                    scalar=upd_a[:, t : t + 1],
                    in1=wh_tiles[t][:], op0=Alu.mult, op1=Alu.add,
                )
                _adh(fi.ins, vidx_inst.ins, sync=False,
                     reason="FMAs after pack-index chain")
                nc.scalar.dma_start(out=whn.ap()[t * P : (t + 1) * P, :],
                                    in_=ot[:])

            # ---- small outputs (back to AT layout for contiguous stores) ----
            pto = ps.tile([C, P], f32, space="PSUM", tag="t16")
            nc.tensor.transpose(pto[:], accn_a[:], id128[:])
            accn_t = sm.tile([C, P], f32)
            nc.scalar.copy(out=accn_t[:], in_=pto[:])
            nc.scalar.dma_start(out=at_view(accn), in_=accn_t[:])

            ptu2 = ps.tile([C, P], f32, space="PSUM", tag="t16")
            nc.tensor.transpose(ptu2[:], u2_a[:], id128[:])
            remn_t = sm.tile([C, P], f32)
            nc.vector.tensor_add(out=remn_t[:], in0=rem_t[:], in1=ptu2[:])
            nc.scalar.dma_start(out=at_view(remn), in_=remn_t[:])

            ptme = ps.tile([C, P], f32, space="PSUM", tag="t16")
            nc.tensor.transpose(ptme[:], me_a[:], id128[:])
            exif = sm.tile([C, P], f32)
            nc.vector.tensor_copy(out=exif[:], in_=exi_t[:])
            exnf = sm.tile([C, P], f32)
            nc.vector.scalar_tensor_tensor(
                out=exnf[:], in0=ptme[:], scalar=updates_plus1, in1=exif[:],
                op0=Alu.mult, op1=Alu.add,
            )
            exn_t = sm.tile([C, P], i32)
            nc.vector.tensor_copy(out=exn_t[:], in_=exnf[:])
            nc.scalar.dma_start(out=at_view(exn), in_=exn_t[:])
            ptm = ps.tile([C, P], f32, space="PSUM", tag="t16")
            nc.tensor.transpose(ptm[:], mc_a[:], id128[:])
            mc8 = sm.tile([C, P], u8)
            nc.vector.tensor_copy(out=mc8[:], in_=ptm[:])
            nc.scalar.dma_start(out=at_view(runn), in_=mc8[:])

    nc.compile()
    return nc


def _consts():
    us16 = np.triu(np.ones((C, C), dtype=np.float32), 1)  # [k,i]=1 iff k<i
    us128 = np.triu(np.ones((P, P), dtype=np.float32), 0)  # [k,i]=1 iff k<=i
    id16 = np.eye(C, dtype=np.float32)
    id128 = np.eye(P, dtype=np.float32)
    return us16, us128, id16, id128


def make_in_maps(x, run, acc_p, weighted_h, remainders, exit_, pad_h, p_w, p_b):
    us16, us128, id16, id128 = _consts()
    pad128 = np.broadcast_to(
        np.asarray(pad_h, dtype=np.float32).reshape(1, H), (P, H)).copy()
    pw128 = np.broadcast_to(
        np.asarray(p_w, dtype=np.float32).reshape(1, H), (P, H)).copy()
    pb128 = np.full((P, 1), np.float32(np.asarray(p_b).reshape(1)[0]),
                    dtype=np.float32)
    in_maps = []
    for b in range(NCORES):
        in_maps.append(
            {
                "x": np.ascontiguousarray(x[b], dtype=np.float32),
                "run8": np.ascontiguousarray(run[b]).astype(np.uint8),
                "accp": np.ascontiguousarray(acc_p[b]).reshape(M).astype(np.float32),
                "wh": np.ascontiguousarray(weighted_h[b], dtype=np.float32),
                "rem": np.ascontiguousarray(remainders[b]).reshape(M).astype(np.float32),
                "exi": np.ascontiguousarray(exit_[b]).reshape(M).astype(np.int32),
                "pad128": pad128,
                "pw128": pw128,
                "pb128": pb128,
                "us16": us16,
                "us128": us128,
                "id16": id16,
                "id128": id128,
            }
        )
    return in_maps


def kernel(x, run, acc_p, weighted_h, remainders, exit_, updates, pad_h, p_w, p_b,
           _want_results_obj=False, _trace=False):
    from concourse.bass_utils import run_bass_kernel_spmd

    x = np.asarray(x)
    run = np.asarray(run)
    acc_p = np.asarray(acc_p)
    weighted_h = np.asarray(weighted_h)
    remainders = np.asarray(remainders)
    exit_ = np.asarray(exit_)

    nc = _build(float(np.asarray(updates)) + 1.0)
    in_maps = make_in_maps(x, run, acc_p, weighted_h, remainders, exit_,
                           pad_h, p_w, p_b)
    res = run_bass_kernel_spmd(nc, in_maps, core_ids=list(range(NCORES)),
                               trace=_trace)

    h_packed = np.stack([res.results[b]["hp"] for b in range(NCORES)])
    whn = np.stack([res.results[b]["whn"] for b in range(NCORES)])
    accn = np.stack([res.results[b]["accn"] for b in range(NCORES)]).reshape(B, M, 1)
    remn = np.stack([res.results[b]["remn"] for b in range(NCORES)]).reshape(B, M, 1)
    runn = np.stack([res.results[b]["runn"] for b in range(NCORES)]).astype(bool)
    exn = np.stack([res.results[b]["exn"] for b in range(NCORES)]).reshape(B, M, 1)
    out = (h_packed, whn, accn, remn, runn, exn.astype(np.int32))
    if _want_results_obj:
        return out, res
    return out


# revision 34
# speedup vs baseline: 1.0790x; 1.0790x over previous
"""Adaptive Computation Time step — Trainium2 Bass kernel.

Pure data parallel over batch B=8 on 8 NeuronCores (one batch row per core).

Per-core layouts (M=2048 tokens, H=1024, P=128, C=16; M == P*C):
  - "AT" layout: SBUF [16, 128], token m at [m // 128, m % 128].
    DRAM [2048] <-> AT is a contiguous 512B-per-partition DMA.
  - "A" layout: token m at [m % 128, m // 128] — matches the big H tile
    [128, 16, 1024] (token = partition, chunk = m // 128) used by the
    per-chunk indirect gathers/scatters, whose index APs are [128,1] columns.
  - AT <-> A is a single PE transpose (via identity matmul).

Algorithm per core:
  rank   = cumsum(run) - 1                  (DVE scan + 16x16 tri matmul)
  H[t]   = x[rank] for running tokens       (indirect row gather; non-running
           rows are OOB-skipped and the chunk is pre-zeroed: h = 0 there,
           exactly like the reference)
  p      = sigmoid(h @ p_w + p_b) * run     (fused mul+accum per chunk + ACT)
  mc     = (acc_p + p < 0.99) & run ; me = run & ~mc       (per half, A layout,
  update = p*mc + (1-acc_p)*me               so chunk FMAs start early)
  weighted_h_new = h*update + weighted_h    (one fused DVE op per chunk)
  c2     = cumsum(mc); dest = mc ? c2-1 : count_new + (m - c2)  (bijection)
  H[~mc rows] = pad_h (copy_predicated), then per-chunk indirect scatter of
  H rows to dest — every h_packed row is written exactly once.
"""

import functools
import sys

sys.path.insert(0, "/opt/trn_rl_repo")

import numpy as np

B, M, H = 8, 2048, 1024
P, C = 128, 16
TH = 0.99
NCORES = 8
OOB = 4096.0  # > M-1 -> skipped by bounds_check


@functools.lru_cache(maxsize=4)
def _build(updates_plus1: float):
    import concourse.bass as bass
    import concourse.tile as tile
    from concourse import bacc, mybir

    f32 = mybir.dt.float32
    i32 = mybir.dt.int32
    u8 = mybir.dt.uint8
    Alu = mybir.AluOpType
    Act = mybir.ActivationFunctionType

    nc = bacc.Bacc("TRN2", target_bir_lowering=False, debug=False)

    x = nc.dram_tensor("x", [M, H], f32, kind="ExternalInput")
    run8 = nc.dram_tensor("run8", [M], u8, kind="ExternalInput")
    accp = nc.dram_tensor("accp", [M], f32, kind="ExternalInput")
    wh = nc.dram_tensor("wh", [M, H], f32, kind="ExternalInput")
    rem = nc.dram_tensor("rem", [M], f32, kind="ExternalInput")
    exi = nc.dram_tensor("exi", [M], i32, kind="ExternalInput")
    pad128_d = nc.dram_tensor("pad128", [P, H], f32, kind="ExternalInput")
    pw128_d = nc.dram_tensor("pw128", [P, H], f32, kind="ExternalInput")
    pb128_d = nc.dram_tensor("pb128", [P, 1], f32, kind="ExternalInput")
    us16_d = nc.dram_tensor("us16", [C, C], f32, kind="ExternalInput")
    us128_d = nc.dram_tensor("us128", [P, P], f32, kind="ExternalInput")
    id16_d = nc.dram_tensor("id16", [C, C], f32, kind="ExternalInput")
    id128_d = nc.dram_tensor("id128", [P, P], f32, kind="ExternalInput")

    hp = nc.dram_tensor("hp", [M, H], f32, kind="ExternalOutput")
    whn = nc.dram_tensor("whn", [M, H], f32, kind="ExternalOutput")
    accn = nc.dram_tensor("accn", [M], f32, kind="ExternalOutput")
    remn = nc.dram_tensor("remn", [M], f32, kind="ExternalOutput")
    runn = nc.dram_tensor("runn", [M], u8, kind="ExternalOutput")
    exn = nc.dram_tensor("exn", [M], i32, kind="ExternalOutput")

    def at_view(dram):  # DRAM [2048] -> [16,128] AT view
        return dram.ap().rearrange("(t p) -> t p", p=P)

    with tile.TileContext(nc) as tc:
        with (
            tc.tile_pool(name="const", bufs=1) as cst,
            tc.tile_pool(name="small", bufs=1) as sm,
            tc.tile_pool(name="big", bufs=1) as big,
            tc.tile_pool(name="whs", bufs=16) as whs,
            tc.tile_pool(name="outs", bufs=4) as outs,
            tc.tile_pool(name="scr", bufs=2) as scr,
            tc.tile_pool(name="ps", bufs=2, space="PSUM") as ps,
            tc.tile_pool(name="ps2", bufs=2, space="PSUM") as ps2,
        ):
            # ---- critical-path loads on sync; the rest on scalar ----
            run8_t = sm.tile([C, P], u8)
            nc.sync.dma_start(out=run8_t[:], in_=at_view(run8))
            id16 = cst.tile([C, C], f32)
            nc.sync.dma_start(out=id16[:], in_=id16_d.ap())
            us16 = cst.tile([C, C], f32)
            nc.sync.dma_start(out=us16[:], in_=us16_d.ap())
            Ht = big.tile([P, C, H], f32)
            nc.scalar.memzero(Ht[:, 0:4, :])
            pad128 = cst.tile([P, H], f32)
            nc.scalar.dma_start(out=pad128[:], in_=pad128_d.ap())
            pw128 = cst.tile([P, H], f32)
            nc.scalar.dma_start(out=pw128[:], in_=pw128_d.ap())
            pb128 = cst.tile([P, 1], f32)
            nc.scalar.dma_start(out=pb128[:], in_=pb128_d.ap())
            accp_t = sm.tile([C, P], f32)
            nc.sync.dma_start(out=accp_t[:], in_=at_view(accp))
            id128 = cst.tile([P, P], f32)
            nc.scalar.dma_start(out=id128[:], in_=id128_d.ap())
            us128 = cst.tile([P, P], f32)
            nc.scalar.dma_start(out=us128[:], in_=us128_d.ap())
            zrow = cst.tile([P, C], f32)
            nc.vector.memset(zrow[:], 0.0)
            rem_t = sm.tile([C, P], f32)
            nc.sync.dma_start(out=rem_t[:], in_=at_view(rem))
            exi_t = sm.tile([C, P], i32)
            nc.sync.dma_start(out=exi_t[:], in_=at_view(exi))
            # remaining big-tile memzero quarters (quarter 0 was issued at
            # the top of the ACT stream; OOB-skipped rows must read 0.0)
            for q in range(1, 4):
                nc.scalar.memzero(Ht[:, q * 4 : (q + 1) * 4, :])

            runf = sm.tile([C, P], f32)
            nc.vector.tensor_copy(out=runf[:], in_=run8_t[:])
            z16 = sm.tile([C, P], f32)
            nc.vector.memset(z16[:], 0.0)

            # ---- cumsum #1: rank, then masked gather index (AT layout) ----
            s1 = sm.tile([C, P], f32)
            nc.vector.tensor_tensor_scan(
                out=s1[:], data0=runf[:], data1=z16[:], initial=0.0,
                op0=Alu.add, op1=Alu.add,
            )
            ex1 = ps2.tile([C, 1], f32, space="PSUM", tag="tmisc")
            nc.tensor.matmul(out=ex1[:], lhsT=us16[:], rhs=s1[:, P - 1 : P],
                             start=True, stop=True)
            c1 = sm.tile([C, P], f32)
            nc.vector.tensor_scalar(
                out=c1[:], in0=s1[:], scalar1=ex1[:], scalar2=1.0,
                op0=Alu.add, op1=Alu.subtract,
            )
            # gidx = run ? max(rank,0) : OOB  ==  (max(rank,0) - OOB)*run + OOB
            g0 = sm.tile([C, P], f32)
            nc.vector.tensor_single_scalar(out=g0[:], in_=c1[:], scalar=0.0,
                                           op=Alu.max)
            g1 = sm.tile([C, P], f32)
            nc.vector.scalar_tensor_tensor(
                out=g1[:], in0=g0[:], scalar=-OOB, in1=runf[:],
                op0=Alu.add, op1=Alu.mult,
            )
            gidx_at = sm.tile([C, P], f32)
            nc.vector.tensor_single_scalar(out=gidx_at[:], in_=g1[:],
                                           scalar=OOB, op=Alu.add)
            ptg = ps.tile([P, C], f32, space="PSUM", tag="t128")
            nc.tensor.transpose(ptg[:], gidx_at[:], id16[:])
            gidx_a = sm.tile([P, C], i32)
            nc.vector.tensor_copy(out=gidx_a[:], in_=ptg[:])

            # A-layout run/accp for the per-half mask math
            ptr_ = ps.tile([P, C], f32, space="PSUM", tag="t128")
            nc.tensor.transpose(ptr_[:], runf[:], id16[:])
            runf_a = sm.tile([P, C], f32)
            nc.scalar.copy(out=runf_a[:], in_=ptr_[:])
            pta = ps.tile([P, C], f32, space="PSUM", tag="t128")
            nc.tensor.transpose(pta[:], accp_t[:], id16[:])
            accp_a = sm.tile([P, C], f32)
            nc.scalar.copy(out=accp_a[:], in_=pta[:])

            # ---- prefill h_packed with pad rows (sequential writes that
            # overlap the read-heavy gather phase); the valid scatter later
            # overwrites the first count_new rows ----
            for t in range(C):
                nc.sync.dma_start(out=hp.ap()[t * P : (t + 1) * P, :],
                                  in_=pad128[:])

            # ---- weighted_h loads (early, deep pipeline) ----
            wh_tiles = []
            for t in range(C):
                wt = whs.tile([P, H], f32, tag="wht")
                nc.sync.dma_start(out=wt[:], in_=wh.ap()[t * P : (t + 1) * P, :])
                wh_tiles.append(wt)

            HALF = C // 2
            pdot = sm.tile([P, C], f32)
            sig_a = sm.tile([P, C], f32)
            p_a = sm.tile([P, C], f32)
            mc_a = sm.tile([P, C], f32)
            me_a = sm.tile([P, C], f32)
            u1_a = sm.tile([P, C], f32)
            u2_a = sm.tile([P, C], f32)
            upd_a = sm.tile([P, C], f32)
            accn_a = sm.tile([P, C], f32)

            def half_math(hf):
                hs = slice(hf * HALF, (hf + 1) * HALF)
                nc.scalar.activation(sig_a[:, hs], pdot[:, hs], Act.Sigmoid,
                                     bias=pb128[:], scale=1.0)
                nc.vector.tensor_mul(out=p_a[:, hs], in0=sig_a[:, hs],
                                     in1=runf_a[:, hs])
                tmp = sm.tile([P, HALF], f32, tag="tmp_a")
                nc.vector.tensor_add(out=tmp[:], in0=accp_a[:, hs],
                                     in1=p_a[:, hs])
                mcf = sm.tile([P, HALF], f32, tag="mcf_a")
                nc.vector.tensor_single_scalar(out=mcf[:], in_=tmp[:],
                                               scalar=TH, op=Alu.is_lt)
                nc.vector.tensor_mul(out=mc_a[:, hs], in0=mcf[:],
                                     in1=runf_a[:, hs])
                nc.vector.tensor_sub(out=me_a[:, hs], in0=runf_a[:, hs],
                                     in1=mc_a[:, hs])
                nc.vector.tensor_mul(out=u1_a[:, hs], in0=p_a[:, hs],
                                     in1=mc_a[:, hs])
                t2 = sm.tile([P, HALF], f32, tag="t2_a")
                nc.vector.tensor_mul(out=t2[:], in0=accp_a[:, hs],
                                     in1=me_a[:, hs])
                nc.vector.tensor_sub(out=u2_a[:, hs], in0=me_a[:, hs],
                                     in1=t2[:])
                nc.vector.tensor_add(out=upd_a[:, hs], in0=u1_a[:, hs],
                                     in1=u2_a[:, hs])
                nc.vector.tensor_add(out=accn_a[:, hs], in0=accp_a[:, hs],
                                     in1=u1_a[:, hs])

            # gathers + pdots chase each other; mask math per half
            for t in range(C):
                nc.gpsimd.indirect_dma_start(
                    out=Ht[:, t, :],
                    out_offset=None,
                    in_=x.ap(),
                    in_offset=bass.IndirectOffsetOnAxis(
                        ap=gidx_a[:, t : t + 1], axis=0),
                    bounds_check=M - 1,
                    oob_is_err=False,
                )
                sc = scr.tile([P, H], f32, tag="scr")
                nc.vector.scalar_tensor_tensor(
                    out=sc[:], in0=Ht[:, t, :], scalar=1.0, in1=pw128[:],
                    op0=Alu.mult, op1=Alu.mult,
                    accum_out=pdot[:, t : t + 1],
                )
                if t == HALF - 1:
                    half_math(0)
            half_math(1)


            # ---- cumsum #2 over mc, entirely in A layout (no transposes
            # on the critical path): partition-prefix via one 128x128
            # triangular matmul, column-prefix via a 16-wide scan on
            # partition 127 + a K=1 broadcast matmul ----
            colpref = ps.tile([P, C], f32, space="PSUM", tag="t128")
            nc.tensor.matmul(out=colpref[:], lhsT=us128[:], rhs=mc_a[:],
                             start=True, stop=True)
            colp_sb = sm.tile([P, C], f32)
            nc.scalar.copy(out=colp_sb[:], in_=colpref[:])
            # column totals to partition 0 (us128 col 127 is all-ones)
            ctot = ps2.tile([1, C], f32, space="PSUM", tag="tb")
            nc.tensor.matmul(out=ctot[:], lhsT=us128[:, P - 1 : P],
                             rhs=mc_a[:], start=True, stop=True)
            srow = sm.tile([1, C], f32)
            nc.vector.tensor_tensor_scan(
                out=srow[:], data0=ctot[:], data1=zrow[0:1, 0:C], initial=0.0,
                op0=Alu.add, op1=Alu.add,
            )
            tb = ps2.tile([P, C], f32, space="PSUM", tag="tb")
            nc.tensor.matmul(out=tb[:], lhsT=us128[0:1, :],
                             rhs=srow[:], start=True, stop=True)
            c2a = sm.tile([P, C], f32)
            nc.vector.tensor_copy(out=c2a[:, 0:1], in_=colp_sb[:, 0:1])
            nc.vector.tensor_add(out=c2a[:, 1:C], in0=colp_sb[:, 1:C],
                                 in1=tb[:, 0 : C - 1])
            # vidx = mc ? c2 - 1 : OOB  ==  (c2 - 1 - OOB)*mc + OOB
            v1 = sm.tile([P, C], f32)
            nc.vector.tensor_single_scalar(out=v1[:], in_=c2a[:],
                                           scalar=-(1.0 + OOB), op=Alu.add)
            v2 = sm.tile([P, C], f32)
            nc.vector.tensor_mul(out=v2[:], in0=v1[:], in1=mc_a[:])
            v3 = sm.tile([P, C], f32)
            nc.vector.tensor_single_scalar(out=v3[:], in_=v2[:], scalar=OOB,
                                           op=Alu.add)
            vidx_a = sm.tile([P, C], i32)
            nc.vector.tensor_copy(out=vidx_a[:], in_=v3[:])

            # ---- pack scatters: valid rows only (non-running rows OOB-skip,
            # the pad prefill remains in the tail slots). Call 0 keeps the
            # Tile dep on the prefill writes; once that completed, later
            # calls are ordered behind it on the same engine, so the WAW
            # chain between scatter calls can be dropped (destinations are
            # distinct packed slots).
            for t in range(C):
                nc.gpsimd.indirect_dma_start(
                    out=hp.ap(),
                    out_offset=bass.IndirectOffsetOnAxis(
                        ap=vidx_a[:, t : t + 1], axis=0),
                    in_=Ht[:, t, :],
                    in_offset=None,
                    bounds_check=M - 1,
                    oob_is_err=False,
                )
                tc.dep_state.clear_tensor_accesses(hp.name)


            # ---- FMAs (weighted_h_new), free-running on DVE ----
            for t in range(C):
                ot = outs.tile([P, H], f32, tag="outt")
                nc.vector.scalar_tensor_tensor(
                    out=ot[:], in0=Ht[:, t, :],
                    scalar=upd_a[:, t : t + 1],
                    in1=wh_tiles[t][:], op0=Alu.mult, op1=Alu.add,
                )
                nc.scalar.dma_start(out=whn.ap()[t * P : (t + 1) * P, :],
                                    in_=ot[:])

            # ---- small outputs (back to AT layout for contiguous stores) ----
            pto = ps.tile([C, P], f32, space="PSUM", tag="t16")
            nc.tensor.transpose(pto[:], accn_a[:], id128[:])
            accn_t = sm.tile([C, P], f32)
            nc.scalar.copy(out=accn_t[:], in_=pto[:])
            nc.scalar.dma_start(out=at_view(accn), in_=accn_t[:])

            ptu2 = ps.tile([C, P], f32, space="PSUM", tag="t16")
            nc.tensor.transpose(ptu2[:], u2_a[:], id128[:])
            remn_t = sm.tile([C, P], f32)
            nc.vector.tensor_add(out=remn_t[:], in0=rem_t[:], in1=ptu2[:])
            nc.scalar.dma_start(out=at_view(remn), in_=remn_t[:])

            ptme = ps.tile([C, P], f32, space="PSUM", tag="t16")
            nc.tensor.transpose(ptme[:], me_a[:], id128[:])
            exif = sm.tile([C, P], f32)
            nc.vector.tensor_copy(out=exif[:], in_=exi_t[:])
            exnf = sm.tile([C, P], f32)
            nc.vector.scalar_tensor_tensor(
                out=exnf[:], in0=ptme[:], scalar=updates_plus1, in1=exif[:],
                op0=Alu.mult, op1=Alu.add,
            )
            exn_t = sm.tile([C, P], i32)
            nc.vector.tensor_copy(out=exn_t[:], in_=exnf[:])
            nc.scalar.dma_start(out=at_view(exn), in_=exn_t[:])
            ptm = ps.tile([C, P], f32, space="PSUM", tag="t16")
            nc.tensor.transpose(ptm[:], mc_a[:], id128[:])
            mc8 = sm.tile([C, P], u8)
            nc.vector.tensor_copy(out=mc8[:], in_=ptm[:])
            nc.scalar.dma_start(out=at_view(runn), in_=mc8[:])

    nc.compile()
    return nc


def _consts():
    us16 = np.triu(np.ones((C, C), dtype=np.float32), 1)  # [k,i]=1 iff k<i
    us128 = np.triu(np.ones((P, P), dtype=np.float32), 0)  # [k,i]=1 iff k<=i
    id16 = np.eye(C, dtype=np.float32)
    id128 = np.eye(P, dtype=np.float32)
    return us16, us128, id16, id128


def make_in_maps(x, run, acc_p, weighted_h, remainders, exit_, pad_h, p_w, p_b):
    us16, us128, id16, id128 = _consts()
    pad128 = np.broadcast_to(
        np.asarray(pad_h, dtype=np.float32).reshape(1, H), (P, H)).copy()
    pw128 = np.broadcast_to(
        np.asarray(p_w, dtype=np.float32).reshape(1, H), (P, H)).copy()
    pb128 = np.full((P, 1), np.float32(np.asarray(p_b).reshape(1)[0]),
                    dtype=np.float32)
    in_maps = []
    for b in range(NCORES):
        in_maps.append(
            {
                "x": np.ascontiguousarray(x[b], dtype=np.float32),
                "run8": np.ascontiguousarray(run[b]).astype(np.uint8),
                "accp": np.ascontiguousarray(acc_p[b]).reshape(M).astype(np.float32),
                "wh": np.ascontiguousarray(weighted_h[b], dtype=np.float32),
                "rem": np.ascontiguousarray(remainders[b]).reshape(M).astype(np.float32),
                "exi": np.ascontiguousarray(exit_[b]).reshape(M).astype(np.int32),
                "pad128": pad128,
                "pw128": pw128,
                "pb128": pb128,
                "us16": us16,
                "us128": us128,
                "id16": id16,
                "id128": id128,
            }
        )
    return in_maps


def kernel(x, run, acc_p, weighted_h, remainders, exit_, updates, pad_h, p_w, p_b,
           _want_results_obj=False, _trace=False):
    from concourse.bass_utils import run_bass_kernel_spmd

    x = np.asarray(x)
    run = np.asarray(run)
    acc_p = np.asarray(acc_p)
    weighted_h = np.asarray(weighted_h)
    remainders = np.asarray(remainders)
    exit_ = np.asarray(exit_)

    nc = _build(float(np.asarray(updates)) + 1.0)
    in_maps = make_in_maps(x, run, acc_p, weighted_h, remainders, exit_,
                           pad_h, p_w, p_b)
    res = run_bass_kernel_spmd(nc, in_maps, core_ids=list(range(NCORES)),
                               trace=_trace)

    h_packed = np.stack([res.results[b]["hp"] for b in range(NCORES)])
    whn = np.stack([res.results[b]["whn"] for b in range(NCORES)])
    accn = np.stack([res.results[b]["accn"] for b in range(NCORES)]).reshape(B, M, 1)
    remn = np.stack([res.results[b]["remn"] for b in range(NCORES)]).reshape(B, M, 1)
    runn = np.stack([res.results[b]["runn"] for b in range(NCORES)]).astype(bool)
    exn = np.stack([res.results[b]["exn"] for b in range(NCORES)]).reshape(B, M, 1)
    out = (h_packed, whn, accn, remn, runn, exn.astype(np.int32))
    if _want_results_obj:
        return out, res
    return out
